# revision 15
# baseline (speedup 1.0000x reference)
"""Trainium2 Bass kernel for FixedPointHGRNAttention.

Reference computation (B=4, T=2048, D=2048):
    x  = round(hs*256)/256
    i  = (x @ w_i) * s_i ; f = sigmoid((x @ w_f) * s_f) ; g = (x @ w_g) * s_g
    i  = (1-f)*i ; h_t = f_t*h_{t-1} + i_t  (scan over T, per channel)
    rms = h * rsqrt(mean(h^2, ch) + eps)
    o  = rms * g_norm_w * silu(g)
    out = round(((o*s_o) @ w_o.T)*256)/256

Sharding: 8 cores = 4 batches x 2 sequence halves. Each core computes its
[b, half] slice end-to-end in transposed [channel, time] layout; the scan
carry h[b, TC-1, :] crosses the half boundary via four batched [128, NE/4]
pair AllReduces (issued as each quarter of phase 1 completes, so each is
hidden behind remaining phase-1/phase-2 matmuls with >=60us slack). Carry
fixups h += Fprod*carry are interleaved into phase 2. No other collectives
needed.

All matmuls run in fp16. The quantized activations (multiples of 1/256,
|x*256| < 2^11) and the ternary weights are exactly representable in fp16,
so the i/f/g and o_proj products are exact (fp32 PSUM accumulation).

The per-timestep rms scale R commutes past the o_proj channel contraction,
so phase 3 multiplies the PSUM output columns by 256*R instead of
rescaling o up front — phase-3 matmuls start without waiting for the
rms reduction, and phase 2 writes o = h*sogn*silu(g) (fp16) in place of
the consumed Fprod buffer.

Engine budget per core (CoreSim): PE 445us busy at ~96% occupancy of a
~461us span; DVE 130us; Act 106us; Pool (collectives) 84us, hidden.
"""
import numpy as np

import concourse.bass as bass
import concourse.mybir as mybir
import concourse.tile as tile
from concourse import bacc
from concourse.bass_utils import run_bass_kernel_spmd

AF = mybir.ActivationFunctionType
OP = mybir.AluOpType
F32 = mybir.dt.float32
F16 = mybir.dt.float16

MAGIC = float(3 << 22)  # 1.5*2^23: float->int round-to-nearest-even trick
B, T, D = 4, 2048, 2048
TC = T // 2         # timesteps per core
NE = D // 128       # output-channel chunks
NK = D // 128       # contraction chunks
KB = 4              # k-chunks batched per weight DMA
NKG = NK // KB
MV = 512            # moving-operand (free dim) block
NTH = TC // MV
EPS = 1e-5

REPLICA_PAIRS = [[0, 1], [2, 3], [4, 5], [6, 7]]


def _build_kernel(dbg=False):
    nc = bacc.Bacc("TRN2", target_bir_lowering=False, debug=False, num_devices=8)
    xT = nc.dram_tensor("xT", [D, TC], F32, kind="ExternalInput").ap()
    wi = nc.dram_tensor("wi", [NE, NKG, 128, KB * 128], F16, kind="ExternalInput").ap()
    wf = nc.dram_tensor("wf", [NE, NKG, 128, KB * 128], F16, kind="ExternalInput").ap()
    wg = nc.dram_tensor("wg", [NE, NKG, 128, KB * 128], F16, kind="ExternalInput").ap()
    wo = nc.dram_tensor("wo", [NE, NKG, 128, KB * 128], F16, kind="ExternalInput").ap()
    si = nc.dram_tensor("si", [128, NE], F32, kind="ExternalInput").ap()
    sf = nc.dram_tensor("sf", [128, NE], F32, kind="ExternalInput").ap()
    sg = nc.dram_tensor("sg", [128, NE], F32, kind="ExternalInput").ap()
    sogn = nc.dram_tensor("sogn", [128, NE], F32, kind="ExternalInput").ap()
    cmc = nc.dram_tensor("cmc", [128, 1], F32, kind="ExternalInput").ap()
    cmu = nc.dram_tensor("cmu", [128, NE], F32, kind="ExternalInput").ap()
    outT = nc.dram_tensor("outT", [D, TC], F32, kind="ExternalOutput").ap()
    dbg_aps = None
    if dbg:
        dbg_aps = {
            n: nc.dram_tensor(n, s, dt, kind="ExternalOutput").ap()
            for n, s, dt in [("dbg_xq", [D, TC], F16), ("dbg_h", [D, TC], F32),
                             ("dbg_f0", [128, TC], F32), ("dbg_ig0", [128, TC], F32),
                             ("dbg_hsw", [D, TC], F32), ("dbg_r", [1, TC], F32),
                             ("dbg_o", [D, TC], F16)]}

    with tile.TileContext(nc) as tc:
        _body(tc, xT, wi, wf, wg, wo, si, sf, sg, sogn, cmc, cmu, outT, dbg_aps)
    nc.compile()
    return nc


def _body(tc, xT, wi, wf, wg, wo, si, sf, sg, sogn, cmc, cmu, outT, dbg_aps=None):
    nc = tc.nc
    from contextlib import ExitStack
    with ExitStack() as ctx:
        singles = ctx.enter_context(tc.tile_pool(name="singles", bufs=1))
        big = ctx.enter_context(tc.tile_pool(name="big", bufs=1))
        work = ctx.enter_context(tc.tile_pool(name="work", bufs=3))
        wpool = ctx.enter_context(tc.tile_pool(name="wpool", bufs=6))
        dram = ctx.enter_context(tc.tile_pool(name="dram", bufs=4, space="DRAM"))

        # constants / scales
        si_sb = singles.tile([128, NE], F32)
        nc.sync.dma_start(out=si_sb[:], in_=si)
        sf_sb = singles.tile([128, NE], F32)
        nc.sync.dma_start(out=sf_sb[:], in_=sf)
        sg_sb = singles.tile([128, NE], F32)
        nc.sync.dma_start(out=sg_sb[:], in_=sg)
        sogn_sb = singles.tile([128, NE], F32)
        nc.sync.dma_start(out=sogn_sb[:], in_=sogn)
        cmc_sb = singles.tile([128, 1], F32)
        nc.sync.dma_start(out=cmc_sb[:], in_=cmc)
        cmu_sb = singles.tile([128, NE], F32)
        nc.sync.dma_start(out=cmu_sb[:], in_=cmu)
        ones_sb = singles.tile([128, 1], F16)
        nc.vector.memset(ones_sb[:], 1.0)
        eps_sb = singles.tile([128, 1], F32)
        nc.vector.memset(eps_sb[:], EPS / 65536.0)
        contribs_sb = singles.tile([128, NE], F32)
        carry_sb = singles.tile([128, NE], F32)

        # persistent big buffers: h (fp32), fo (fp16: forget-products for the
        # carry fixup in phase 1/2, then reused in-place for o = h*sogn*sw in
        # phase 2/3). x^T lives in its own pool, freed at phase 3.
        h_all = big.tile([128, NE, TC], F32)
        fo_pool_cm = tc.tile_pool(name="fo_pool", bufs=1)
        fo_pool = fo_pool_cm.__enter__()
        fo_all = fo_pool.tile([128, NE, TC], F16)
        xq_pool_cm = tc.tile_pool(name="xq_pool", bufs=1)
        xq_pool = xq_pool_cm.__enter__()
        xq_all = xq_pool.tile([128, NK, TC], F16)

        # ---- phase 0: load + quantize x^T ----
        # (x loads go on the gpsimd queue so the phase-1 weight DMAs on the
        # sync queue are not stuck behind 8 MB of input traffic)
        for k in range(NK):
            xraw = work.tile([128, TC], F32, tag="wka")
            nc.gpsimd.dma_start(out=xraw[:], in_=xT[k * 128:(k + 1) * 128, :])
            tmp = work.tile([128, TC], F32, tag="wkb")
            nc.scalar.activation(tmp[:], xraw[:], AF.Copy, bias=MAGIC, scale=256.0)
            nc.vector.tensor_scalar(xq_all[:, k, :], tmp[:], MAGIC, 1.0 / 256.0,
                                    OP.subtract, OP.mult)

        # ---- phase 1: f/i matmuls, gating, scans, carry contribs ----
        with tc.tile_pool(name="ps1", bufs=2, space="PSUM") as ps1:
            for e in range(NE):
                ps_f = ps1.tile([128, TC], F32, tag="ps_f")
                ps_i = ps1.tile([128, TC], F32, tag="ps_i")
                for kg in range(NKG):
                    wf_t = wpool.tile([128, KB * 128], F16, tag="wf")
                    nc.sync.dma_start(out=wf_t[:], in_=wf[e, kg])
                    wi_t = wpool.tile([128, KB * 128], F16, tag="wi")
                    nc.sync.dma_start(out=wi_t[:], in_=wi[e, kg])
                    for kl in range(KB):
                        k = kg * KB + kl
                        st, sp = (k == 0), (k == NK - 1)
                        for th in range(NTH):
                            nc.tensor.matmul(ps_f[:, th * MV:(th + 1) * MV],
                                             wf_t[:, kl * 128:(kl + 1) * 128],
                                             xq_all[:, k, th * MV:(th + 1) * MV],
                                             start=st, stop=sp)
                        for th in range(NTH):
                            nc.tensor.matmul(ps_i[:, th * MV:(th + 1) * MV],
                                             wi_t[:, kl * 128:(kl + 1) * 128],
                                             xq_all[:, k, th * MV:(th + 1) * MV],
                                             start=st, stop=sp)
                f_sb = work.tile([128, TC], F32, tag="wkb")
                nc.scalar.activation(f_sb[:], ps_f[:], AF.Sigmoid,
                                     scale=sf_sb[:, e:e + 1])
                omf = work.tile([128, TC], F32, tag="wka")
                nc.scalar.activation(omf[:], f_sb[:], AF.Copy, bias=1.0, scale=-1.0)
                ig = work.tile([128, TC], F32, tag="wkc")
                nc.vector.scalar_tensor_tensor(ig[:], ps_i[:], si_sb[:, e:e + 1],
                                               omf[:], OP.mult, OP.mult)
                if dbg_aps is not None and e == 0:
                    nc.sync.dma_start(out=dbg_aps["dbg_f0"], in_=f_sb[:])
                    nc.sync.dma_start(out=dbg_aps["dbg_ig0"], in_=ig[:])
                h_e = h_all[:, e, :]
                nc.vector.tensor_tensor_scan(h_e, f_sb[:], ig[:], 0.0,
                                             OP.mult, OP.add)
                nc.vector.tensor_tensor_scan(fo_all[:, e, :], f_sb[:], f_sb[:],
                                             1.0, OP.mult, OP.bypass)
                nc.vector.tensor_mul(contribs_sb[:, e:e + 1],
                                     h_e[:, TC - 1:TC], cmc_sb[:])
                if (e + 1) % (NE // 4) == 0:
                    # batched pair AllReduce for this quarter's carries; all
                    # but the last are issued mid-phase-1, and the last has
                    # 12 chunks of phase-2 matmul slack before its fixups
                    lo = e + 1 - NE // 4
                    hi = e + 1
                    cc_in = dram.tile([128, NE // 4], F32, tag="cc_in")
                    nc.gpsimd.dma_start(out=cc_in[:], in_=contribs_sb[:, lo:hi])
                    cc_out = dram.tile([128, NE // 4], F32, tag="cc_out")
                    nc.gpsimd.collective_compute(
                        "AllReduce", OP.add, replica_groups=REPLICA_PAIRS,
                        ins=[cc_in.opt()], outs=[cc_out.opt()])
                    nc.gpsimd.dma_start(out=carry_sb[:, lo:hi], in_=cc_out[:])
                    nc.vector.tensor_mul(carry_sb[:, lo:hi], carry_sb[:, lo:hi],
                                         cmu_sb[:, lo:hi])

        if dbg_aps is not None:
            for k in range(NK):
                nc.sync.dma_start(out=dbg_aps["dbg_xq"][k * 128:(k + 1) * 128, :],
                                  in_=xq_all[:, k, :])

        # ---- phase 2: carry fixups, g matmuls, silu, h^2 column-sum,
        #      h = h*sogn*sw ----
        with tc.tile_pool(name="ps2", bufs=2, space="PSUM") as ps2, \
             tc.tile_pool(name="pss", bufs=1, space="PSUM") as pss:
            ss = []
            for th in range(NTH):
                ss_th = pss.tile([1, MV], F32, tag=f"ss{th}")
                ss.append(ss_th)
            for e in range(NE):
                ps_g = ps2.tile([128, TC], F32, tag="ps_g")
                for kg in range(NKG):
                    wg_t = wpool.tile([128, KB * 128], F16, tag="wg")
                    nc.sync.dma_start(out=wg_t[:], in_=wg[e, kg])
                    for kl in range(KB):
                        k = kg * KB + kl
                        for th in range(NTH):
                            nc.tensor.matmul(ps_g[:, th * MV:(th + 1) * MV],
                                             wg_t[:, kl * 128:(kl + 1) * 128],
                                             xq_all[:, k, th * MV:(th + 1) * MV],
                                             start=(k == 0), stop=(k == NK - 1))
                h_e = h_all[:, e, :]
                # carry fixup: h += Fprod * (carry * is_upper_half)
                nc.vector.scalar_tensor_tensor(h_e, fo_all[:, e, :],
                                               carry_sb[:, e:e + 1], h_e,
                                               OP.mult, OP.add)
                sq = work.tile([128, TC], F16, tag="sq")
                nc.scalar.activation(sq[:], h_e, AF.Square)
                for th in range(NTH):
                    nc.tensor.matmul(ss[th][:], ones_sb[:],
                                     sq[:, th * MV:(th + 1) * MV],
                                     start=(e == 0), stop=(e == NE - 1))
                sw = work.tile([128, TC], F32, tag="wkb")
                nc.scalar.activation(sw[:], ps_g[:], AF.Silu,
                                     scale=sg_sb[:, e:e + 1])
                # o = h*sogn*sw overwrites the Fprod slot (fp16); the rms
                # scale R commutes past the o_proj contraction and is applied
                # per-column on the PSUM output in phase 3
                nc.vector.scalar_tensor_tensor(fo_all[:, e, :], h_e,
                                               sogn_sb[:, e:e + 1],
                                               sw[:], OP.mult, OP.mult)

            if dbg_aps is not None:
                for e in range(NE):
                    nc.sync.dma_start(out=dbg_aps["dbg_h"][e * 128:(e + 1) * 128, :],
                                      in_=h_all[:, e, :])

            # 256 * rms_inv = 1/sqrt((mean + eps)/65536), broadcast across
            # partitions (the 256 pre-scales the output for fixed rounding)
            r_row = singles.tile([1, TC], F32)
            for th in range(NTH):
                nc.scalar.activation(r_row[:, th * MV:(th + 1) * MV], ss[th][:],
                                     AF.Sqrt, bias=eps_sb[:1, 0:1],
                                     scale=1.0 / (D * 65536.0))
            nc.vector.reciprocal(r_row[:], r_row[:])
            r_dram = dram.tile([1, TC], F32, tag="r_dram")
            nc.sync.dma_start(out=r_dram[:], in_=r_row[:])
            R_sb = singles.tile([128, TC], F32)
            nc.sync.dma_start(out=R_sb[:], in_=r_dram[:].to_broadcast([128, TC]))

        if dbg_aps is not None:
            nc.sync.dma_start(out=dbg_aps["dbg_r"], in_=r_row[:])
            for e in range(NE):
                nc.sync.dma_start(out=dbg_aps["dbg_hsw"][e * 128:(e + 1) * 128, :],
                                  in_=h_all[:, e, :])

        # ---- phase 3: out^T = wo.T @ o, then *R256 per column + final round ----
        xq_pool_cm.__exit__(None, None, None)
        if dbg_aps is not None:
            for e in range(NE):
                nc.sync.dma_start(out=dbg_aps["dbg_o"][e * 128:(e + 1) * 128, :],
                                  in_=fo_all[:, e, :])
        with tc.tile_pool(name="ps3", bufs=2, space="PSUM") as ps3:
            for d in range(NE):
                ps_o = ps3.tile([128, TC], F32, tag="ps_o")
                for eg in range(NKG):
                    wo_t = wpool.tile([128, KB * 128], F16, tag="wo")
                    nc.sync.dma_start(out=wo_t[:], in_=wo[d, eg])
                    for el in range(KB):
                        e = eg * KB + el
                        for th in range(NTH):
                            nc.tensor.matmul(ps_o[:, th * MV:(th + 1) * MV],
                                             wo_t[:, el * 128:(el + 1) * 128],
                                             fo_all[:, e, th * MV:(th + 1) * MV],
                                             start=(e == 0), stop=(e == NE - 1))
                t0 = work.tile([128, TC], F32, tag="wkc")
                nc.vector.tensor_tensor(t0[:], ps_o[:], R_sb[:], OP.mult)
                t1 = work.tile([128, TC], F32, tag="wka")
                nc.scalar.activation(t1[:], t0[:], AF.Copy, bias=MAGIC)
                ot = work.tile([128, TC], F32, tag="wkb")
                nc.vector.tensor_scalar(ot[:], t1[:], MAGIC, 1.0 / 256.0,
                                        OP.subtract, OP.mult)
                nc.sync.dma_start(out=outT[d * 128:(d + 1) * 128, :], in_=ot[:])
        fo_pool_cm.__exit__(None, None, None)


_NC_CACHE = None


def _get_nc():
    global _NC_CACHE
    if _NC_CACHE is None:
        _NC_CACHE = _build_kernel()
    return _NC_CACHE


def _retile(w):
    # [R, C] -> [NC_col, NKG, 128, KB*128] fp16, where
    # out[c, kg, p, kl*128 + m] = w[(kg*KB+kl)*128 + p, c*128 + m].
    # Slice [:, kl*128:(kl+1)*128] of tile (c, kg) is the lhsT for
    # contraction chunk k = kg*KB+kl and output-column chunk c.
    g = w.astype(np.float16).reshape(NKG, KB, 128, NE, 128)
    return np.ascontiguousarray(g.transpose(3, 0, 2, 1, 4).reshape(NE, NKG, 128, KB * 128))


def _scale_cols(s):
    # [D] -> [128, NE] with column e = s[e*128:(e+1)*128]
    return np.ascontiguousarray(s.reshape(NE, 128).T)


def _make_in_maps(inputs):
    hidden_states = np.asarray(inputs["hidden_states"], dtype=np.float32)
    wi_t = _retile(np.asarray(inputs["w_i"], np.float32))
    wf_t = _retile(np.asarray(inputs["w_f"], np.float32))
    wg_t = _retile(np.asarray(inputs["w_g"], np.float32))
    # o_proj: kernel reads wo[d, eg] batches; slice el is the lhsT
    # (w_o.T)[(eg*KB+el)*128 : .. , d*128 : ..]
    wo_t = _retile(np.ascontiguousarray(np.asarray(inputs["w_o"], np.float32).T))
    si_c = _scale_cols(np.asarray(inputs["s_i"], np.float32))
    sf_c = _scale_cols(np.asarray(inputs["s_f"], np.float32))
    sg_c = _scale_cols(np.asarray(inputs["s_g"], np.float32))
    sogn_c = _scale_cols(np.asarray(inputs["s_o"], np.float32)
                         * np.asarray(inputs["g_norm_w"], np.float32))

    in_maps = []
    for c in range(8):
        b, half = divmod(c, 2)
        xT = np.ascontiguousarray(
            hidden_states[b, half * TC:(half + 1) * TC, :].T)
        in_maps.append({
            "xT": xT, "wi": wi_t, "wf": wf_t, "wg": wg_t, "wo": wo_t,
            "si": si_c, "sf": sf_c, "sg": sg_c, "sogn": sogn_c,
            "cmc": np.full((128, 1), 1.0 - half, np.float32),
            "cmu": np.full((128, NE), float(half), np.float32),
        })
    return in_maps


def kernel(hidden_states, w_i, w_f, w_g, w_o, s_i, s_f, s_g, s_o, g_norm_w):
    nc = _get_nc()
    in_maps = _make_in_maps(dict(
        hidden_states=hidden_states, w_i=w_i, w_f=w_f, w_g=w_g, w_o=w_o,
        s_i=s_i, s_f=s_f, s_g=s_g, s_o=s_o, g_norm_w=g_norm_w))
    res = run_bass_kernel_spmd(nc, in_maps, list(range(8)))
    out = np.empty((B, T, D), np.float32)
    for c in range(8):
        b, half = divmod(c, 2)
        out[b, half * TC:(half + 1) * TC, :] = res.results[c]["outT"].T
    return out


# revision 19
# speedup vs baseline: 1.0051x; 1.0051x over previous
"""Trainium2 Bass kernel for FixedPointHGRNAttention.

Reference computation (B=4, T=2048, D=2048):
    x  = round(hs*256)/256
    i  = (x @ w_i) * s_i ; f = sigmoid((x @ w_f) * s_f) ; g = (x @ w_g) * s_g
    i  = (1-f)*i ; h_t = f_t*h_{t-1} + i_t  (scan over T, per channel)
    rms = h * rsqrt(mean(h^2, ch) + eps)
    o  = rms * g_norm_w * silu(g)
    out = round(((o*s_o) @ w_o.T)*256)/256

Sharding: 8 cores = 4 batches x 2 sequence halves. Each core computes its
[b, half] slice end-to-end in transposed [channel, time] layout; the scan
carry h[b, TC-1, :] crosses the half boundary via four batched [128, NE/4]
pair AllReduces (issued as each quarter of phase 1 completes, so each is
hidden behind remaining phase-1/phase-2 matmuls with >=60us slack). Carry
fixups h += Fprod*carry are interleaved into phase 2. No other collectives
needed.

All matmuls run in fp16. The quantized activations (multiples of 1/256,
|x*256| < 2^11) and the ternary weights are exactly representable in fp16,
so the i/f/g and o_proj products are exact (fp32 PSUM accumulation).

The per-timestep rms scale R commutes past the o_proj channel contraction,
so phase 3 multiplies the PSUM output columns by 256*R instead of
rescaling o up front — phase-3 matmuls start without waiting for the
rms reduction, and phase 2 writes o = h*sogn*silu(g) (fp16) in place of
the consumed Fprod buffer.

Engine budget per core (CoreSim): PE 445us busy at ~96% occupancy of a
~461us span; DVE 130us; Act 106us; Pool (collectives) 84us, hidden.
"""
import numpy as np

import concourse.bass as bass
import concourse.mybir as mybir
import concourse.tile as tile
from concourse import bacc
from concourse.bass_utils import run_bass_kernel_spmd

AF = mybir.ActivationFunctionType
OP = mybir.AluOpType
F32 = mybir.dt.float32
F16 = mybir.dt.float16

MAGIC = float(3 << 22)  # 1.5*2^23: float->int round-to-nearest-even trick
B, T, D = 4, 2048, 2048
TC = T // 2         # timesteps per core
NE = D // 128       # output-channel chunks
NK = D // 128       # contraction chunks
KB = 4              # k-chunks batched per weight DMA
NKG = NK // KB
MV = 512            # moving-operand (free dim) block
NTH = TC // MV
EPS = 1e-5

REPLICA_PAIRS = [[0, 1], [2, 3], [4, 5], [6, 7]]


def _build_kernel(dbg=False):
    nc = bacc.Bacc("TRN2", target_bir_lowering=False, debug=False, num_devices=8)
    xT = nc.dram_tensor("xT", [D, TC], F32, kind="ExternalInput").ap()
    wi = nc.dram_tensor("wi", [NE, NKG, 128, KB * 128], F16, kind="ExternalInput").ap()
    wf = nc.dram_tensor("wf", [NE, NKG, 128, KB * 128], F16, kind="ExternalInput").ap()
    wg = nc.dram_tensor("wg", [NE, NKG, 128, KB * 128], F16, kind="ExternalInput").ap()
    wo = nc.dram_tensor("wo", [NE, NKG, 128, KB * 128], F16, kind="ExternalInput").ap()
    si = nc.dram_tensor("si", [128, NE], F32, kind="ExternalInput").ap()
    sf = nc.dram_tensor("sf", [128, NE], F32, kind="ExternalInput").ap()
    sg = nc.dram_tensor("sg", [128, NE], F32, kind="ExternalInput").ap()
    sogn = nc.dram_tensor("sogn", [128, NE], F32, kind="ExternalInput").ap()
    cmc = nc.dram_tensor("cmc", [128, 1], F32, kind="ExternalInput").ap()
    cmu = nc.dram_tensor("cmu", [128, NE], F32, kind="ExternalInput").ap()
    outT = nc.dram_tensor("outT", [D, TC], F32, kind="ExternalOutput").ap()
    dbg_aps = None
    if dbg:
        dbg_aps = {
            n: nc.dram_tensor(n, s, dt, kind="ExternalOutput").ap()
            for n, s, dt in [("dbg_xq", [D, TC], F16), ("dbg_h", [D, TC], F32),
                             ("dbg_f0", [128, TC], F32), ("dbg_ig0", [128, TC], F32),
                             ("dbg_hsw", [D, TC], F32), ("dbg_r", [1, TC], F32),
                             ("dbg_o", [D, TC], F16)]}

    with tile.TileContext(nc) as tc:
        _body(tc, xT, wi, wf, wg, wo, si, sf, sg, sogn, cmc, cmu, outT, dbg_aps)
    nc.compile()
    return nc


def _body(tc, xT, wi, wf, wg, wo, si, sf, sg, sogn, cmc, cmu, outT, dbg_aps=None):
    nc = tc.nc
    from contextlib import ExitStack
    with ExitStack() as ctx:
        singles = ctx.enter_context(tc.tile_pool(name="singles", bufs=1))
        big = ctx.enter_context(tc.tile_pool(name="big", bufs=1))
        work = ctx.enter_context(tc.tile_pool(name="work", bufs=3))
        wpool = ctx.enter_context(tc.tile_pool(name="wpool", bufs=6))
        dram = ctx.enter_context(tc.tile_pool(name="dram", bufs=4, space="DRAM"))

        # constants / scales
        si_sb = singles.tile([128, NE], F32)
        nc.sync.dma_start(out=si_sb[:], in_=si)
        sf_sb = singles.tile([128, NE], F32)
        nc.sync.dma_start(out=sf_sb[:], in_=sf)
        sg_sb = singles.tile([128, NE], F32)
        nc.sync.dma_start(out=sg_sb[:], in_=sg)
        sogn_sb = singles.tile([128, NE], F32)
        nc.sync.dma_start(out=sogn_sb[:], in_=sogn)
        cmc_sb = singles.tile([128, 1], F32)
        nc.sync.dma_start(out=cmc_sb[:], in_=cmc)
        cmu_sb = singles.tile([128, NE], F32)
        nc.sync.dma_start(out=cmu_sb[:], in_=cmu)
        nsf_sb = singles.tile([128, NE], F32)
        nc.vector.tensor_scalar(nsf_sb[:], sf_sb[:], -1.0, 0.0, OP.mult, OP.add)
        ones_sb = singles.tile([128, 1], F16)
        nc.vector.memset(ones_sb[:], 1.0)
        eps_sb = singles.tile([128, 1], F32)
        nc.vector.memset(eps_sb[:], EPS / 65536.0)
        contribs_sb = singles.tile([128, NE], F32)
        carry_sb = singles.tile([128, NE], F32)

        # persistent big buffers: h (fp32), fo (fp16: forget-products for the
        # carry fixup in phase 1/2, then reused in-place for o = h*sogn*sw in
        # phase 2/3). x^T lives in its own pool, freed at phase 3.
        h_all = big.tile([128, NE, TC], F32)
        fo_pool_cm = tc.tile_pool(name="fo_pool", bufs=1)
        fo_pool = fo_pool_cm.__enter__()
        fo_all = fo_pool.tile([128, NE, TC], F16)
        xq_pool_cm = tc.tile_pool(name="xq_pool", bufs=1)
        xq_pool = xq_pool_cm.__enter__()
        xq_all = xq_pool.tile([128, NK, TC], F16)

        # ---- phase 0: load + quantize x^T ----
        # (x loads go on the gpsimd queue so the phase-1 weight DMAs on the
        # sync queue are not stuck behind 8 MB of input traffic; quantize in
        # MV-wide halves so the first matmul operand is ready sooner)
        for k in range(NK):
            xraw = work.tile([128, TC], F32, tag="wka")
            nc.gpsimd.dma_start(out=xraw[:], in_=xT[k * 128:(k + 1) * 128, :])
            for th in range(NTH):
                sl = slice(th * MV, (th + 1) * MV)
                tmp = work.tile([128, MV], F32, tag="wkb")
                nc.scalar.activation(tmp[:], xraw[:, sl], AF.Copy,
                                     bias=MAGIC, scale=256.0)
                nc.vector.tensor_scalar(xq_all[:, k, sl], tmp[:], MAGIC,
                                        1.0 / 256.0, OP.subtract, OP.mult)

        # ---- phase 1: f/i matmuls, gating, scans, carry contribs ----
        with tc.tile_pool(name="ps1", bufs=2, space="PSUM") as ps1:
            for e in range(NE):
                ps_f = ps1.tile([128, TC], F32, tag="ps_f")
                ps_i = ps1.tile([128, TC], F32, tag="ps_i")
                for kg in range(NKG):
                    wf_t = wpool.tile([128, KB * 128], F16, tag="wf")
                    nc.sync.dma_start(out=wf_t[:], in_=wf[e, kg])
                    wi_t = wpool.tile([128, KB * 128], F16, tag="wi")
                    nc.sync.dma_start(out=wi_t[:], in_=wi[e, kg])
                    for kl in range(KB):
                        k = kg * KB + kl
                        st, sp = (k == 0), (k == NK - 1)
                        for th in range(NTH):
                            nc.tensor.matmul(ps_f[:, th * MV:(th + 1) * MV],
                                             wf_t[:, kl * 128:(kl + 1) * 128],
                                             xq_all[:, k, th * MV:(th + 1) * MV],
                                             start=st, stop=sp)
                        for th in range(NTH):
                            nc.tensor.matmul(ps_i[:, th * MV:(th + 1) * MV],
                                             wi_t[:, kl * 128:(kl + 1) * 128],
                                             xq_all[:, k, th * MV:(th + 1) * MV],
                                             start=st, stop=sp)
                # 1-f = sigmoid(-z) reads PSUM directly (not serialized
                # behind f), shortening the chain that holds the ps1 banks
                omf = work.tile([128, TC], F32, tag="wka")
                nc.scalar.activation(omf[:], ps_f[:], AF.Sigmoid,
                                     scale=nsf_sb[:, e:e + 1])
                f_sb = work.tile([128, TC], F32, tag="wkb")
                nc.scalar.activation(f_sb[:], ps_f[:], AF.Sigmoid,
                                     scale=sf_sb[:, e:e + 1])
                ig = work.tile([128, TC], F32, tag="wkc")
                nc.vector.scalar_tensor_tensor(ig[:], ps_i[:], si_sb[:, e:e + 1],
                                               omf[:], OP.mult, OP.mult)
                if dbg_aps is not None and e == 0:
                    nc.sync.dma_start(out=dbg_aps["dbg_f0"], in_=f_sb[:])
                    nc.sync.dma_start(out=dbg_aps["dbg_ig0"], in_=ig[:])
                h_e = h_all[:, e, :]
                nc.vector.tensor_tensor_scan(h_e, f_sb[:], ig[:], 0.0,
                                             OP.mult, OP.add)
                nc.vector.tensor_tensor_scan(fo_all[:, e, :], f_sb[:], f_sb[:],
                                             1.0, OP.mult, OP.bypass)
                nc.vector.tensor_mul(contribs_sb[:, e:e + 1],
                                     h_e[:, TC - 1:TC], cmc_sb[:])
                if (e + 1) % (NE // 4) == 0:
                    # batched pair AllReduce for this quarter's carries; all
                    # but the last are issued mid-phase-1, and the last has
                    # 12 chunks of phase-2 matmul slack before its fixups
                    lo = e + 1 - NE // 4
                    hi = e + 1
                    cc_in = dram.tile([128, NE // 4], F32, tag="cc_in")
                    nc.gpsimd.dma_start(out=cc_in[:], in_=contribs_sb[:, lo:hi])
                    cc_out = dram.tile([128, NE // 4], F32, tag="cc_out")
                    nc.gpsimd.collective_compute(
                        "AllReduce", OP.add, replica_groups=REPLICA_PAIRS,
                        ins=[cc_in.opt()], outs=[cc_out.opt()])
                    nc.gpsimd.dma_start(out=carry_sb[:, lo:hi], in_=cc_out[:])
                    nc.vector.tensor_mul(carry_sb[:, lo:hi], carry_sb[:, lo:hi],
                                         cmu_sb[:, lo:hi])

        if dbg_aps is not None:
            for k in range(NK):
                nc.sync.dma_start(out=dbg_aps["dbg_xq"][k * 128:(k + 1) * 128, :],
                                  in_=xq_all[:, k, :])

        # ---- phase 2: carry fixups, g matmuls, silu, h^2 column-sum,
        #      h = h*sogn*sw ----
        with tc.tile_pool(name="ps2", bufs=2, space="PSUM") as ps2, \
             tc.tile_pool(name="pss", bufs=1, space="PSUM") as pss:
            ss = []
            for th in range(NTH):
                ss_th = pss.tile([1, MV], F32, tag=f"ss{th}")
                ss.append(ss_th)
            for e in range(NE):
                ps_g = ps2.tile([128, TC], F32, tag="ps_g")
                for kg in range(NKG):
                    wg_t = wpool.tile([128, KB * 128], F16, tag="wg")
                    nc.sync.dma_start(out=wg_t[:], in_=wg[e, kg])
                    for kl in range(KB):
                        k = kg * KB + kl
                        for th in range(NTH):
                            nc.tensor.matmul(ps_g[:, th * MV:(th + 1) * MV],
                                             wg_t[:, kl * 128:(kl + 1) * 128],
                                             xq_all[:, k, th * MV:(th + 1) * MV],
                                             start=(k == 0), stop=(k == NK - 1))
                h_e = h_all[:, e, :]
                # carry fixup: h += Fprod * (carry * is_upper_half)
                nc.vector.scalar_tensor_tensor(h_e, fo_all[:, e, :],
                                               carry_sb[:, e:e + 1], h_e,
                                               OP.mult, OP.add)
                sq = work.tile([128, TC], F16, tag="sq")
                nc.scalar.activation(sq[:], h_e, AF.Square)
                for th in range(NTH):
                    nc.tensor.matmul(ss[th][:], ones_sb[:],
                                     sq[:, th * MV:(th + 1) * MV],
                                     start=(e == 0), stop=(e == NE - 1))
                sw = work.tile([128, TC], F32, tag="wkb")
                nc.scalar.activation(sw[:], ps_g[:], AF.Silu,
                                     scale=sg_sb[:, e:e + 1])
                # o = h*sogn*sw overwrites the Fprod slot (fp16); the rms
                # scale R commutes past the o_proj contraction and is applied
                # per-column on the PSUM output in phase 3
                nc.vector.scalar_tensor_tensor(fo_all[:, e, :], h_e,
                                               sogn_sb[:, e:e + 1],
                                               sw[:], OP.mult, OP.mult)

            if dbg_aps is not None:
                for e in range(NE):
                    nc.sync.dma_start(out=dbg_aps["dbg_h"][e * 128:(e + 1) * 128, :],
                                      in_=h_all[:, e, :])

            # 256 * rms_inv = 1/sqrt((mean + eps)/65536), broadcast across
            # partitions (the 256 pre-scales the output for fixed rounding)
            r_row = singles.tile([1, TC], F32)
            for th in range(NTH):
                nc.scalar.activation(r_row[:, th * MV:(th + 1) * MV], ss[th][:],
                                     AF.Sqrt, bias=eps_sb[:1, 0:1],
                                     scale=1.0 / (D * 65536.0))
            nc.vector.reciprocal(r_row[:], r_row[:])
            r_dram = dram.tile([1, TC], F32, tag="r_dram")
            nc.sync.dma_start(out=r_dram[:], in_=r_row[:])
            R_sb = singles.tile([128, TC], F32)
            nc.sync.dma_start(out=R_sb[:], in_=r_dram[:].to_broadcast([128, TC]))

        if dbg_aps is not None:
            nc.sync.dma_start(out=dbg_aps["dbg_r"], in_=r_row[:])
            for e in range(NE):
                nc.sync.dma_start(out=dbg_aps["dbg_hsw"][e * 128:(e + 1) * 128, :],
                                  in_=h_all[:, e, :])

        # ---- phase 3: out^T = wo.T @ o, then *R256 per column + final round ----
        xq_pool_cm.__exit__(None, None, None)
        if dbg_aps is not None:
            for e in range(NE):
                nc.sync.dma_start(out=dbg_aps["dbg_o"][e * 128:(e + 1) * 128, :],
                                  in_=fo_all[:, e, :])
        with tc.tile_pool(name="ps3", bufs=2, space="PSUM") as ps3:
            for d in range(NE):
                ps_o = ps3.tile([128, TC], F32, tag="ps_o")
                for eg in range(NKG):
                    wo_t = wpool.tile([128, KB * 128], F16, tag="wo")
                    nc.sync.dma_start(out=wo_t[:], in_=wo[d, eg])
                    for el in range(KB):
                        e = eg * KB + el
                        for th in range(NTH):
                            nc.tensor.matmul(ps_o[:, th * MV:(th + 1) * MV],
                                             wo_t[:, el * 128:(el + 1) * 128],
                                             fo_all[:, e, th * MV:(th + 1) * MV],
                                             start=(e == 0), stop=(e == NE - 1))
                # post-process per MV-wide half so the rounding chain and
                # output DMA pipeline with the tail of the matmul stream
                for th in range(NTH):
                    sl = slice(th * MV, (th + 1) * MV)
                    t0 = work.tile([128, MV], F32, tag="wkc")
                    nc.vector.tensor_tensor(t0[:], ps_o[:, sl], R_sb[:, sl],
                                            OP.mult)
                    t1 = work.tile([128, MV], F32, tag="wka")
                    nc.scalar.activation(t1[:], t0[:], AF.Copy, bias=MAGIC)
                    ot = work.tile([128, MV], F32, tag="wkb")
                    nc.vector.tensor_scalar(ot[:], t1[:], MAGIC, 1.0 / 256.0,
                                            OP.subtract, OP.mult)
                    nc.sync.dma_start(out=outT[d * 128:(d + 1) * 128, sl],
                                      in_=ot[:])
        fo_pool_cm.__exit__(None, None, None)


_NC_CACHE = None


def _get_nc():
    global _NC_CACHE
    if _NC_CACHE is None:
        _NC_CACHE = _build_kernel()
    return _NC_CACHE


def _retile(w):
    # [R, C] -> [NC_col, NKG, 128, KB*128] fp16, where
    # out[c, kg, p, kl*128 + m] = w[(kg*KB+kl)*128 + p, c*128 + m].
    # Slice [:, kl*128:(kl+1)*128] of tile (c, kg) is the lhsT for
    # contraction chunk k = kg*KB+kl and output-column chunk c.
    g = w.astype(np.float16).reshape(NKG, KB, 128, NE, 128)
    return np.ascontiguousarray(g.transpose(3, 0, 2, 1, 4).reshape(NE, NKG, 128, KB * 128))


def _scale_cols(s):
    # [D] -> [128, NE] with column e = s[e*128:(e+1)*128]
    return np.ascontiguousarray(s.reshape(NE, 128).T)


def _make_in_maps(inputs):
    hidden_states = np.asarray(inputs["hidden_states"], dtype=np.float32)
    wi_t = _retile(np.asarray(inputs["w_i"], np.float32))
    wf_t = _retile(np.asarray(inputs["w_f"], np.float32))
    wg_t = _retile(np.asarray(inputs["w_g"], np.float32))
    # o_proj: kernel reads wo[d, eg] batches; slice el is the lhsT
    # (w_o.T)[(eg*KB+el)*128 : .. , d*128 : ..]
    wo_t = _retile(np.ascontiguousarray(np.asarray(inputs["w_o"], np.float32).T))
    si_c = _scale_cols(np.asarray(inputs["s_i"], np.float32))
    sf_c = _scale_cols(np.asarray(inputs["s_f"], np.float32))
    sg_c = _scale_cols(np.asarray(inputs["s_g"], np.float32))
    sogn_c = _scale_cols(np.asarray(inputs["s_o"], np.float32)
                         * np.asarray(inputs["g_norm_w"], np.float32))

    in_maps = []
    for c in range(8):
        b, half = divmod(c, 2)
        xT = np.ascontiguousarray(
            hidden_states[b, half * TC:(half + 1) * TC, :].T)
        in_maps.append({
            "xT": xT, "wi": wi_t, "wf": wf_t, "wg": wg_t, "wo": wo_t,
            "si": si_c, "sf": sf_c, "sg": sg_c, "sogn": sogn_c,
            "cmc": np.full((128, 1), 1.0 - half, np.float32),
            "cmu": np.full((128, NE), float(half), np.float32),
        })
    return in_maps


def kernel(hidden_states, w_i, w_f, w_g, w_o, s_i, s_f, s_g, s_o, g_norm_w):
    nc = _get_nc()
    in_maps = _make_in_maps(dict(
        hidden_states=hidden_states, w_i=w_i, w_f=w_f, w_g=w_g, w_o=w_o,
        s_i=s_i, s_f=s_f, s_g=s_g, s_o=s_o, g_norm_w=g_norm_w))
    res = run_bass_kernel_spmd(nc, in_maps, list(range(8)))
    out = np.empty((B, T, D), np.float32)
    for c in range(8):
        b, half = divmod(c, 2)
        out[b, half * TC:(half + 1) * TC, :] = res.results[c]["outT"].T
    return out


# revision 23
# speedup vs baseline: 1.0067x; 1.0016x over previous
"""Trainium2 Bass kernel for FixedPointHGRNAttention.

Reference computation (B=4, T=2048, D=2048):
    x  = round(hs*256)/256
    i  = (x @ w_i) * s_i ; f = sigmoid((x @ w_f) * s_f) ; g = (x @ w_g) * s_g
    i  = (1-f)*i ; h_t = f_t*h_{t-1} + i_t  (scan over T, per channel)
    rms = h * rsqrt(mean(h^2, ch) + eps)
    o  = rms * g_norm_w * silu(g)
    out = round(((o*s_o) @ w_o.T)*256)/256

Sharding: 8 cores = 4 batches x 2 sequence halves. Each core computes its
[b, half] slice end-to-end in transposed [channel, time] layout; the scan
carry h[b, TC-1, :] crosses the half boundary via four batched [128, NE/4]
pair AllReduces (issued as each quarter of phase 1 completes, so each is
hidden behind remaining phase-1/phase-2 matmuls with >=60us slack). Carry
fixups h += Fprod*carry are interleaved into phase 2. No other collectives
needed.

All matmuls run in fp16. The quantized activations (multiples of 1/256,
|x*256| < 2^11) and the ternary weights are exactly representable in fp16,
so the i/f/g and o_proj products are exact (fp32 PSUM accumulation).

The per-timestep rms scale R commutes past the o_proj channel contraction,
so phase 3 multiplies the PSUM output columns by 256*R instead of
rescaling o up front — phase-3 matmuls start without waiting for the
rms reduction, and phase 2 writes o = h*sogn*silu(g) (fp16) in place of
the consumed Fprod buffer.

Engine budget per core (CoreSim): PE 445us busy at ~96% occupancy of a
~461us span; DVE 130us; Act 106us; Pool (collectives) 84us, hidden.
"""
import numpy as np

import concourse.bass as bass
import concourse.mybir as mybir
import concourse.tile as tile
from concourse import bacc
from concourse.bass_utils import run_bass_kernel_spmd

AF = mybir.ActivationFunctionType
OP = mybir.AluOpType
F32 = mybir.dt.float32
F16 = mybir.dt.float16

MAGIC = float(3 << 22)  # 1.5*2^23: float->int round-to-nearest-even trick
B, T, D = 4, 2048, 2048
TC = T // 2         # timesteps per core
NE = D // 128       # output-channel chunks
NK = D // 128       # contraction chunks
KB = 4              # k-chunks batched per weight DMA
NKG = NK // KB
MV = 512            # moving-operand (free dim) block
NTH = TC // MV
EPS = 1e-5

REPLICA_PAIRS = [[0, 1], [2, 3], [4, 5], [6, 7]]


def _build_kernel(dbg=False):
    nc = bacc.Bacc("TRN2", target_bir_lowering=False, debug=False, num_devices=8)
    xT = nc.dram_tensor("xT", [D, TC], F32, kind="ExternalInput").ap()
    wi = nc.dram_tensor("wi", [NE, NKG, 128, KB * 128], F16, kind="ExternalInput").ap()
    wf = nc.dram_tensor("wf", [NE, NKG, 128, KB * 128], F16, kind="ExternalInput").ap()
    wg = nc.dram_tensor("wg", [NE, NKG, 128, KB * 128], F16, kind="ExternalInput").ap()
    wo = nc.dram_tensor("wo", [NE, NKG, 128, KB * 128], F16, kind="ExternalInput").ap()
    si = nc.dram_tensor("si", [128, NE], F32, kind="ExternalInput").ap()
    sf = nc.dram_tensor("sf", [128, NE], F32, kind="ExternalInput").ap()
    sg = nc.dram_tensor("sg", [128, NE], F32, kind="ExternalInput").ap()
    sogn = nc.dram_tensor("sogn", [128, NE], F32, kind="ExternalInput").ap()
    cmc = nc.dram_tensor("cmc", [128, 1], F32, kind="ExternalInput").ap()
    cmu = nc.dram_tensor("cmu", [128, NE], F32, kind="ExternalInput").ap()
    outT = nc.dram_tensor("outT", [D, TC], F32, kind="ExternalOutput").ap()
    dbg_aps = None
    if dbg:
        dbg_aps = {
            n: nc.dram_tensor(n, s, dt, kind="ExternalOutput").ap()
            for n, s, dt in [("dbg_xq", [D, TC], F16), ("dbg_h", [D, TC], F32),
                             ("dbg_f0", [128, TC], F32), ("dbg_ig0", [128, TC], F32),
                             ("dbg_hsw", [D, TC], F32), ("dbg_r", [1, TC], F32),
                             ("dbg_o", [D, TC], F16)]}

    with tile.TileContext(nc) as tc:
        _body(tc, xT, wi, wf, wg, wo, si, sf, sg, sogn, cmc, cmu, outT, dbg_aps)
    nc.compile()
    return nc


def _body(tc, xT, wi, wf, wg, wo, si, sf, sg, sogn, cmc, cmu, outT, dbg_aps=None):
    nc = tc.nc
    from contextlib import ExitStack
    with ExitStack() as ctx:
        singles = ctx.enter_context(tc.tile_pool(name="singles", bufs=1))
        big = ctx.enter_context(tc.tile_pool(name="big", bufs=1))
        work = ctx.enter_context(tc.tile_pool(name="work", bufs=3))
        wpool = ctx.enter_context(tc.tile_pool(name="wpool", bufs=6))
        dram = ctx.enter_context(tc.tile_pool(name="dram", bufs=4, space="DRAM"))

        # constants / scales
        si_sb = singles.tile([128, NE], F32)
        nc.sync.dma_start(out=si_sb[:], in_=si)
        sf_sb = singles.tile([128, NE], F32)
        nc.sync.dma_start(out=sf_sb[:], in_=sf)
        sg_sb = singles.tile([128, NE], F32)
        nc.sync.dma_start(out=sg_sb[:], in_=sg)
        sogn_sb = singles.tile([128, NE], F32)
        nc.sync.dma_start(out=sogn_sb[:], in_=sogn)
        cmc_sb = singles.tile([128, 1], F32)
        nc.sync.dma_start(out=cmc_sb[:], in_=cmc)
        cmu_sb = singles.tile([128, NE], F32)
        nc.sync.dma_start(out=cmu_sb[:], in_=cmu)
        nsf_sb = singles.tile([128, NE], F32)
        nc.vector.tensor_scalar(nsf_sb[:], sf_sb[:], -1.0, 0.0, OP.mult, OP.add)
        ones_sb = singles.tile([128, 1], F16)
        nc.vector.memset(ones_sb[:], 1.0)
        eps_sb = singles.tile([128, 1], F32)
        nc.vector.memset(eps_sb[:], EPS / 65536.0)
        contribs_sb = singles.tile([128, NE], F32)
        carry_sb = singles.tile([128, NE], F32)

        # persistent big buffers: h (fp32), fo (fp16: forget-products for the
        # carry fixup in phase 1/2, then reused in-place for o = h*sogn*sw in
        # phase 2/3). x^T lives in its own pool, freed at phase 3.
        h_all = big.tile([128, NE, TC], F32)
        fo_pool_cm = tc.tile_pool(name="fo_pool", bufs=1)
        fo_pool = fo_pool_cm.__enter__()
        fo_all = fo_pool.tile([128, NE, TC], F16)
        xq_pool_cm = tc.tile_pool(name="xq_pool", bufs=1)
        xq_pool = xq_pool_cm.__enter__()
        xq_all = xq_pool.tile([128, NK, TC], F16)

        # ---- phase 0: load + quantize x^T ----
        # (x loads go on the gpsimd queue so the phase-1 weight DMAs on the
        # sync queue are not stuck behind 8 MB of input traffic; quantize in
        # MV-wide halves so the first matmul operand is ready sooner)
        for k in range(NK):
            xraw = work.tile([128, TC], F32, tag="wka")
            nc.gpsimd.dma_start(out=xraw[:], in_=xT[k * 128:(k + 1) * 128, :])
            for th in range(NTH):
                sl = slice(th * MV, (th + 1) * MV)
                tmp = work.tile([128, MV], F32, tag="wkb")
                nc.vector.tensor_scalar(tmp[:], xraw[:, sl], 256.0, MAGIC,
                                        OP.mult, OP.add)
                nc.vector.tensor_scalar(xq_all[:, k, sl], tmp[:], MAGIC,
                                        1.0 / 256.0, OP.subtract, OP.mult)

        # ---- phase 1: f/i matmuls, gating, scans, carry contribs ----
        with tc.tile_pool(name="ps1", bufs=2, space="PSUM") as ps1:
            for e in range(NE):
                ps_f = ps1.tile([128, TC], F32, tag="ps_f")
                ps_i = ps1.tile([128, TC], F32, tag="ps_i")
                for kg in range(NKG):
                    wf_t = wpool.tile([128, KB * 128], F16, tag="wf")
                    nc.sync.dma_start(out=wf_t[:], in_=wf[e, kg])
                    wi_t = wpool.tile([128, KB * 128], F16, tag="wi")
                    nc.sync.dma_start(out=wi_t[:], in_=wi[e, kg])
                    for kl in range(KB):
                        k = kg * KB + kl
                        st, sp = (k == 0), (k == NK - 1)
                        for th in range(NTH):
                            nc.tensor.matmul(ps_f[:, th * MV:(th + 1) * MV],
                                             wf_t[:, kl * 128:(kl + 1) * 128],
                                             xq_all[:, k, th * MV:(th + 1) * MV],
                                             start=st, stop=sp)
                        for th in range(NTH):
                            nc.tensor.matmul(ps_i[:, th * MV:(th + 1) * MV],
                                             wi_t[:, kl * 128:(kl + 1) * 128],
                                             xq_all[:, k, th * MV:(th + 1) * MV],
                                             start=st, stop=sp)
                # 1-f = sigmoid(-z) reads PSUM directly (not serialized
                # behind f), shortening the chain that holds the ps1 banks
                omf = work.tile([128, TC], F32, tag="wka")
                nc.scalar.activation(omf[:], ps_f[:], AF.Sigmoid,
                                     scale=nsf_sb[:, e:e + 1])
                f_sb = work.tile([128, TC], F32, tag="wkb")
                nc.scalar.activation(f_sb[:], ps_f[:], AF.Sigmoid,
                                     scale=sf_sb[:, e:e + 1])
                ig = work.tile([128, TC], F32, tag="wkc")
                nc.vector.scalar_tensor_tensor(ig[:], ps_i[:], si_sb[:, e:e + 1],
                                               omf[:], OP.mult, OP.mult)
                if dbg_aps is not None and e == 0:
                    nc.sync.dma_start(out=dbg_aps["dbg_f0"], in_=f_sb[:])
                    nc.sync.dma_start(out=dbg_aps["dbg_ig0"], in_=ig[:])
                h_e = h_all[:, e, :]
                nc.vector.tensor_tensor_scan(h_e, f_sb[:], ig[:], 0.0,
                                             OP.mult, OP.add)
                nc.vector.tensor_tensor_scan(fo_all[:, e, :], f_sb[:], f_sb[:],
                                             1.0, OP.mult, OP.bypass)
                nc.vector.tensor_mul(contribs_sb[:, e:e + 1],
                                     h_e[:, TC - 1:TC], cmc_sb[:])
                if (e + 1) % (NE // 4) == 0:
                    # batched pair AllReduce for this quarter's carries; all
                    # but the last are issued mid-phase-1, and the last has
                    # 12 chunks of phase-2 matmul slack before its fixups
                    lo = e + 1 - NE // 4
                    hi = e + 1
                    cc_in = dram.tile([128, NE // 4], F32, tag="cc_in")
                    nc.gpsimd.dma_start(out=cc_in[:], in_=contribs_sb[:, lo:hi])
                    cc_out = dram.tile([128, NE // 4], F32, tag="cc_out")
                    nc.gpsimd.collective_compute(
                        "AllReduce", OP.add, replica_groups=REPLICA_PAIRS,
                        ins=[cc_in.opt()], outs=[cc_out.opt()])
                    nc.gpsimd.dma_start(out=carry_sb[:, lo:hi], in_=cc_out[:])
                    nc.vector.tensor_mul(carry_sb[:, lo:hi], carry_sb[:, lo:hi],
                                         cmu_sb[:, lo:hi])

        if dbg_aps is not None:
            for k in range(NK):
                nc.sync.dma_start(out=dbg_aps["dbg_xq"][k * 128:(k + 1) * 128, :],
                                  in_=xq_all[:, k, :])

        # ---- phase 2: carry fixups, g matmuls, silu, h^2 column-sum,
        #      h = h*sogn*sw ----
        with tc.tile_pool(name="ps2", bufs=2, space="PSUM") as ps2, \
             tc.tile_pool(name="pss", bufs=1, space="PSUM") as pss:
            ss = []
            for th in range(NTH):
                ss_th = pss.tile([1, MV], F32, tag=f"ss{th}")
                ss.append(ss_th)
            for e in range(NE):
                ps_g = ps2.tile([128, TC], F32, tag="ps_g")
                for kg in range(NKG):
                    wg_t = wpool.tile([128, KB * 128], F16, tag="wg")
                    nc.sync.dma_start(out=wg_t[:], in_=wg[e, kg])
                    for kl in range(KB):
                        k = kg * KB + kl
                        for th in range(NTH):
                            nc.tensor.matmul(ps_g[:, th * MV:(th + 1) * MV],
                                             wg_t[:, kl * 128:(kl + 1) * 128],
                                             xq_all[:, k, th * MV:(th + 1) * MV],
                                             start=(k == 0), stop=(k == NK - 1))
                h_e = h_all[:, e, :]
                # carry fixup: h += Fprod * (carry * is_upper_half)
                nc.vector.scalar_tensor_tensor(h_e, fo_all[:, e, :],
                                               carry_sb[:, e:e + 1], h_e,
                                               OP.mult, OP.add)
                sq = work.tile([128, TC], F16, tag="sq")
                nc.scalar.activation(sq[:], h_e, AF.Square)
                for th in range(NTH):
                    nc.tensor.matmul(ss[th][:], ones_sb[:],
                                     sq[:, th * MV:(th + 1) * MV],
                                     start=(e == 0), stop=(e == NE - 1))
                sw = work.tile([128, TC], F32, tag="wkb")
                nc.scalar.activation(sw[:], ps_g[:], AF.Silu,
                                     scale=sg_sb[:, e:e + 1])
                # o = h*sogn*sw overwrites the Fprod slot (fp16); the rms
                # scale R commutes past the o_proj contraction and is applied
                # per-column on the PSUM output in phase 3
                nc.vector.scalar_tensor_tensor(fo_all[:, e, :], h_e,
                                               sogn_sb[:, e:e + 1],
                                               sw[:], OP.mult, OP.mult)

            if dbg_aps is not None:
                for e in range(NE):
                    nc.sync.dma_start(out=dbg_aps["dbg_h"][e * 128:(e + 1) * 128, :],
                                      in_=h_all[:, e, :])

            # 256 * rms_inv = 1/sqrt((mean + eps)/65536), broadcast across
            # partitions (the 256 pre-scales the output for fixed rounding)
            r_row = singles.tile([1, TC], F32)
            for th in range(NTH):
                nc.scalar.activation(r_row[:, th * MV:(th + 1) * MV], ss[th][:],
                                     AF.Sqrt, bias=eps_sb[:1, 0:1],
                                     scale=1.0 / (D * 65536.0))
            nc.vector.reciprocal(r_row[:], r_row[:])
            r_dram = dram.tile([1, TC], F32, tag="r_dram")
            nc.sync.dma_start(out=r_dram[:], in_=r_row[:])
            R_sb = singles.tile([128, TC], F32)
            nc.sync.dma_start(out=R_sb[:], in_=r_dram[:].to_broadcast([128, TC]))

        if dbg_aps is not None:
            nc.sync.dma_start(out=dbg_aps["dbg_r"], in_=r_row[:])
            for e in range(NE):
                nc.sync.dma_start(out=dbg_aps["dbg_hsw"][e * 128:(e + 1) * 128, :],
                                  in_=h_all[:, e, :])

        # ---- phase 3: out^T = wo.T @ o, then *R256 per column + final round ----
        xq_pool_cm.__exit__(None, None, None)
        if dbg_aps is not None:
            for e in range(NE):
                nc.sync.dma_start(out=dbg_aps["dbg_o"][e * 128:(e + 1) * 128, :],
                                  in_=fo_all[:, e, :])
        with tc.tile_pool(name="ps3", bufs=2, space="PSUM") as ps3:
            for d in range(NE):
                ps_o = ps3.tile([128, TC], F32, tag="ps_o")
                for eg in range(NKG):
                    wo_t = wpool.tile([128, KB * 128], F16, tag="wo")
                    nc.sync.dma_start(out=wo_t[:], in_=wo[d, eg])
                    for el in range(KB):
                        e = eg * KB + el
                        for th in range(NTH):
                            nc.tensor.matmul(ps_o[:, th * MV:(th + 1) * MV],
                                             wo_t[:, el * 128:(el + 1) * 128],
                                             fo_all[:, e, th * MV:(th + 1) * MV],
                                             start=(e == 0), stop=(e == NE - 1))
                # post-process per MV-wide half, entirely on the DVE queue
                # (plus a DVE-issued store) so the drain chain after the last
                # matmul crosses no engine boundary. The device emits
                # round(v*256); the exact *1/256 happens in the host gather.
                for th in range(NTH):
                    sl = slice(th * MV, (th + 1) * MV)
                    t0 = work.tile([128, MV], F32, tag="wkc")
                    nc.vector.tensor_tensor(t0[:], ps_o[:, sl], R_sb[:, sl],
                                            OP.mult)
                    ot = work.tile([128, MV], F32, tag="wkb")
                    nc.vector.tensor_scalar(ot[:], t0[:], MAGIC, MAGIC,
                                            OP.add, OP.subtract)
                    nc.scalar.dma_start(out=outT[d * 128:(d + 1) * 128, sl],
                                        in_=ot[:])
        fo_pool_cm.__exit__(None, None, None)


_NC_CACHE = None


def _get_nc():
    global _NC_CACHE
    if _NC_CACHE is None:
        _NC_CACHE = _build_kernel()
    return _NC_CACHE


def _retile(w):
    # [R, C] -> [NC_col, NKG, 128, KB*128] fp16, where
    # out[c, kg, p, kl*128 + m] = w[(kg*KB+kl)*128 + p, c*128 + m].
    # Slice [:, kl*128:(kl+1)*128] of tile (c, kg) is the lhsT for
    # contraction chunk k = kg*KB+kl and output-column chunk c.
    g = w.astype(np.float16).reshape(NKG, KB, 128, NE, 128)
    return np.ascontiguousarray(g.transpose(3, 0, 2, 1, 4).reshape(NE, NKG, 128, KB * 128))


def _scale_cols(s):
    # [D] -> [128, NE] with column e = s[e*128:(e+1)*128]
    return np.ascontiguousarray(s.reshape(NE, 128).T)


def _make_in_maps(inputs):
    hidden_states = np.asarray(inputs["hidden_states"], dtype=np.float32)
    wi_t = _retile(np.asarray(inputs["w_i"], np.float32))
    wf_t = _retile(np.asarray(inputs["w_f"], np.float32))
    wg_t = _retile(np.asarray(inputs["w_g"], np.float32))
    # o_proj: kernel reads wo[d, eg] batches; slice el is the lhsT
    # (w_o.T)[(eg*KB+el)*128 : .. , d*128 : ..]
    wo_t = _retile(np.ascontiguousarray(np.asarray(inputs["w_o"], np.float32).T))
    si_c = _scale_cols(np.asarray(inputs["s_i"], np.float32))
    sf_c = _scale_cols(np.asarray(inputs["s_f"], np.float32))
    sg_c = _scale_cols(np.asarray(inputs["s_g"], np.float32))
    sogn_c = _scale_cols(np.asarray(inputs["s_o"], np.float32)
                         * np.asarray(inputs["g_norm_w"], np.float32))

    in_maps = []
    for c in range(8):
        b, half = divmod(c, 2)
        xT = np.ascontiguousarray(
            hidden_states[b, half * TC:(half + 1) * TC, :].T)
        in_maps.append({
            "xT": xT, "wi": wi_t, "wf": wf_t, "wg": wg_t, "wo": wo_t,
            "si": si_c, "sf": sf_c, "sg": sg_c, "sogn": sogn_c,
            "cmc": np.full((128, 1), 1.0 - half, np.float32),
            "cmu": np.full((128, NE), float(half), np.float32),
        })
    return in_maps


def kernel(hidden_states, w_i, w_f, w_g, w_o, s_i, s_f, s_g, s_o, g_norm_w):
    nc = _get_nc()
    in_maps = _make_in_maps(dict(
        hidden_states=hidden_states, w_i=w_i, w_f=w_f, w_g=w_g, w_o=w_o,
        s_i=s_i, s_f=s_f, s_g=s_g, s_o=s_o, g_norm_w=g_norm_w))
    res = run_bass_kernel_spmd(nc, in_maps, list(range(8)))
    out = np.empty((B, T, D), np.float32)
    for c in range(8):
        b, half = divmod(c, 2)
        # device emits round(v*256); the *1/256 here is exact (power of two)
        out[b, half * TC:(half + 1) * TC, :] = res.results[c]["outT"].T * np.float32(1 / 256)
    return out


# revision 26
# speedup vs baseline: 1.0142x; 1.0075x over previous
"""Trainium2 Bass kernel for FixedPointHGRNAttention.

Reference computation (B=4, T=2048, D=2048):
    x  = round(hs*256)/256
    i  = (x @ w_i) * s_i ; f = sigmoid((x @ w_f) * s_f) ; g = (x @ w_g) * s_g
    i  = (1-f)*i ; h_t = f_t*h_{t-1} + i_t  (scan over T, per channel)
    rms = h * rsqrt(mean(h^2, ch) + eps)
    o  = rms * g_norm_w * silu(g)
    out = round(((o*s_o) @ w_o.T)*256)/256

Sharding: 8 cores = 4 batches x 2 sequence halves. Each core computes its
[b, half] slice end-to-end in transposed [channel, time] layout; the scan
carry h[b, TC-1, :] crosses the half boundary via four batched [128, NE/4]
pair AllReduces (issued as each quarter of phase 1 completes, so each is
hidden behind remaining phase-1/phase-2 matmuls with >=60us slack). Carry
fixups h += Fprod*carry are interleaved into phase 2. No other collectives
needed.

All matmuls run in fp16. The quantized activations (multiples of 1/256,
|x*256| < 2^11) and the ternary weights are exactly representable in fp16,
so the i/f/g and o_proj products are exact (fp32 PSUM accumulation).

The per-timestep rms scale R commutes past the o_proj channel contraction,
so phase 3 multiplies the PSUM output columns by 256*R instead of
rescaling o up front — phase-3 matmuls start without waiting for the
rms reduction, and phase 2 writes o = h*sogn*silu(g) (fp16) in place of
the consumed Fprod buffer.

Engine budget per core (CoreSim): PE 445us busy at ~96% occupancy of a
~461us span; DVE 130us; Act 106us; Pool (collectives) 84us, hidden.
"""
import numpy as np

import concourse.bass as bass
import concourse.mybir as mybir
import concourse.tile as tile
from concourse import bacc
from concourse.bass_utils import run_bass_kernel_spmd

AF = mybir.ActivationFunctionType
OP = mybir.AluOpType
F32 = mybir.dt.float32
F16 = mybir.dt.float16

MAGIC = float(3 << 22)  # 1.5*2^23: float->int round-to-nearest-even trick
B, T, D = 4, 2048, 2048
TC = T // 2         # timesteps per core
NE = D // 128       # output-channel chunks
NK = D // 128       # contraction chunks
KB = 4              # k-chunks batched per weight DMA
NKG = NK // KB
MV = 512            # moving-operand (free dim) block
NTH = TC // MV
EPS = 1e-5

REPLICA_PAIRS = [[0, 1], [2, 3], [4, 5], [6, 7]]


def _build_kernel(dbg=False):
    nc = bacc.Bacc("TRN2", target_bir_lowering=False, debug=False, num_devices=8)
    xT = nc.dram_tensor("xT", [D, TC], F32, kind="ExternalInput").ap()
    wi = nc.dram_tensor("wi", [NE, NKG, 128, KB * 128], F16, kind="ExternalInput").ap()
    wf = nc.dram_tensor("wf", [NE, NKG, 128, KB * 128], F16, kind="ExternalInput").ap()
    wg = nc.dram_tensor("wg", [NE, NKG, 128, KB * 128], F16, kind="ExternalInput").ap()
    wo = nc.dram_tensor("wo", [NE, NKG, 128, KB * 128], F16, kind="ExternalInput").ap()
    si = nc.dram_tensor("si", [128, NE], F32, kind="ExternalInput").ap()
    sf = nc.dram_tensor("sf", [128, NE], F32, kind="ExternalInput").ap()
    sg = nc.dram_tensor("sg", [128, NE], F32, kind="ExternalInput").ap()
    sogn = nc.dram_tensor("sogn", [128, NE], F32, kind="ExternalInput").ap()
    cmc = nc.dram_tensor("cmc", [128, 1], F32, kind="ExternalInput").ap()
    cmu = nc.dram_tensor("cmu", [128, NE], F32, kind="ExternalInput").ap()
    outT = nc.dram_tensor("outT", [D, TC], F32, kind="ExternalOutput").ap()
    dbg_aps = None
    if dbg:
        dbg_aps = {
            n: nc.dram_tensor(n, s, dt, kind="ExternalOutput").ap()
            for n, s, dt in [("dbg_xq", [D, TC], F16), ("dbg_h", [D, TC], F32),
                             ("dbg_f0", [128, TC], F32), ("dbg_ig0", [128, TC], F32),
                             ("dbg_hsw", [D, TC], F32), ("dbg_r", [1, TC], F32),
                             ("dbg_o", [D, TC], F16)]}

    with tile.TileContext(nc) as tc:
        _body(tc, xT, wi, wf, wg, wo, si, sf, sg, sogn, cmc, cmu, outT, dbg_aps)
    nc.compile()
    return nc


def _body(tc, xT, wi, wf, wg, wo, si, sf, sg, sogn, cmc, cmu, outT, dbg_aps=None):
    nc = tc.nc
    from contextlib import ExitStack
    with ExitStack() as ctx:
        singles = ctx.enter_context(tc.tile_pool(name="singles", bufs=1))
        big = ctx.enter_context(tc.tile_pool(name="big", bufs=1))
        work = ctx.enter_context(tc.tile_pool(name="work", bufs=3))
        wpool = ctx.enter_context(tc.tile_pool(name="wpool", bufs=6))
        dram = ctx.enter_context(tc.tile_pool(name="dram", bufs=4, space="DRAM"))

        # constants / scales
        si_sb = singles.tile([128, NE], F32)
        nc.sync.dma_start(out=si_sb[:], in_=si)
        sf_sb = singles.tile([128, NE], F32)
        nc.sync.dma_start(out=sf_sb[:], in_=sf)
        sg_sb = singles.tile([128, NE], F32)
        nc.sync.dma_start(out=sg_sb[:], in_=sg)
        sogn_sb = singles.tile([128, NE], F32)
        nc.sync.dma_start(out=sogn_sb[:], in_=sogn)
        cmc_sb = singles.tile([128, 1], F32)
        nc.sync.dma_start(out=cmc_sb[:], in_=cmc)
        cmu_sb = singles.tile([128, NE], F32)
        nc.sync.dma_start(out=cmu_sb[:], in_=cmu)
        nsf_sb = singles.tile([128, NE], F32)
        nc.vector.tensor_scalar(nsf_sb[:], sf_sb[:], -1.0, 0.0, OP.mult, OP.add)
        ones_sb = singles.tile([128, 1], F16)
        nc.vector.memset(ones_sb[:], 1.0)
        eps_sb = singles.tile([128, 1], F32)
        nc.vector.memset(eps_sb[:], EPS / 65536.0)
        contribs_sb = singles.tile([128, NE], F32)
        carry_sb = singles.tile([128, NE], F32)

        # persistent big buffers: h (fp32), fo (fp16: forget-products for the
        # carry fixup in phase 1/2, then reused in-place for o = h*sogn*sw in
        # phase 2/3). x^T lives in its own pool, freed at phase 3.
        h_all = big.tile([128, NE, TC], F32)
        fo_pool_cm = tc.tile_pool(name="fo_pool", bufs=1)
        fo_pool = fo_pool_cm.__enter__()
        fo_all = fo_pool.tile([128, NE, TC], F16)
        xq_pool_cm = tc.tile_pool(name="xq_pool", bufs=1)
        xq_pool = xq_pool_cm.__enter__()
        xq_all = xq_pool.tile([128, NK, TC], F16)

        # ---- phase 0: load + quantize x^T ----
        # (x loads go on the gpsimd queue so the phase-1 weight DMAs on the
        # sync queue are not stuck behind 8 MB of input traffic; quantize in
        # MV-wide halves so the first matmul operand is ready sooner)
        for k in range(NK):
            xraw = work.tile([128, TC], F32, tag="wka")
            if k == 0:
                # halve the first transfer so quantization (and the first
                # matmul behind it) starts at half-DMA granularity
                for th in range(NTH):
                    sl = slice(th * MV, (th + 1) * MV)
                    nc.gpsimd.dma_start(out=xraw[:, sl],
                                        in_=xT[k * 128:(k + 1) * 128, sl])
            else:
                nc.gpsimd.dma_start(out=xraw[:], in_=xT[k * 128:(k + 1) * 128, :])
            for th in range(NTH):
                sl = slice(th * MV, (th + 1) * MV)
                tmp = work.tile([128, MV], F32, tag="wkb")
                nc.vector.tensor_scalar(tmp[:], xraw[:, sl], 256.0, MAGIC,
                                        OP.mult, OP.add)
                nc.vector.tensor_scalar(xq_all[:, k, sl], tmp[:], MAGIC,
                                        1.0 / 256.0, OP.subtract, OP.mult)

        # ---- phase 1: f/i matmuls, gating, scans, carry contribs ----
        with tc.tile_pool(name="ps1", bufs=2, space="PSUM") as ps1:
            for e in range(NE):
                ps_f = ps1.tile([128, TC], F32, tag="ps_f")
                ps_i = ps1.tile([128, TC], F32, tag="ps_i")
                for kg in range(NKG):
                    wf_t = wpool.tile([128, KB * 128], F16, tag="wf")
                    nc.sync.dma_start(out=wf_t[:], in_=wf[e, kg])
                    wi_t = wpool.tile([128, KB * 128], F16, tag="wi")
                    nc.sync.dma_start(out=wi_t[:], in_=wi[e, kg])
                    for kl in range(KB):
                        k = kg * KB + kl
                        st, sp = (k == 0), (k == NK - 1)
                        for th in range(NTH):
                            nc.tensor.matmul(ps_f[:, th * MV:(th + 1) * MV],
                                             wf_t[:, kl * 128:(kl + 1) * 128],
                                             xq_all[:, k, th * MV:(th + 1) * MV],
                                             start=st, stop=sp)
                        for th in range(NTH):
                            nc.tensor.matmul(ps_i[:, th * MV:(th + 1) * MV],
                                             wi_t[:, kl * 128:(kl + 1) * 128],
                                             xq_all[:, k, th * MV:(th + 1) * MV],
                                             start=st, stop=sp)
                # 1-f = sigmoid(-z) reads PSUM directly (not serialized
                # behind f), shortening the chain that holds the ps1 banks
                omf = work.tile([128, TC], F32, tag="wka")
                nc.scalar.activation(omf[:], ps_f[:], AF.Sigmoid,
                                     scale=nsf_sb[:, e:e + 1])
                f_sb = work.tile([128, TC], F32, tag="wkb")
                nc.scalar.activation(f_sb[:], ps_f[:], AF.Sigmoid,
                                     scale=sf_sb[:, e:e + 1])
                ig = work.tile([128, TC], F32, tag="wkc")
                nc.vector.scalar_tensor_tensor(ig[:], ps_i[:], si_sb[:, e:e + 1],
                                               omf[:], OP.mult, OP.mult)
                if dbg_aps is not None and e == 0:
                    nc.sync.dma_start(out=dbg_aps["dbg_f0"], in_=f_sb[:])
                    nc.sync.dma_start(out=dbg_aps["dbg_ig0"], in_=ig[:])
                h_e = h_all[:, e, :]
                nc.vector.tensor_tensor_scan(h_e, f_sb[:], ig[:], 0.0,
                                             OP.mult, OP.add)
                nc.vector.tensor_tensor_scan(fo_all[:, e, :], f_sb[:], f_sb[:],
                                             1.0, OP.mult, OP.bypass)
                nc.vector.tensor_mul(contribs_sb[:, e:e + 1],
                                     h_e[:, TC - 1:TC], cmc_sb[:])
                if (e + 1) % (NE // 4) == 0:
                    # batched pair AllReduce for this quarter's carries; all
                    # but the last are issued mid-phase-1, and the last has
                    # 12 chunks of phase-2 matmul slack before its fixups
                    lo = e + 1 - NE // 4
                    hi = e + 1
                    cc_in = dram.tile([128, NE // 4], F32, tag="cc_in")
                    nc.gpsimd.dma_start(out=cc_in[:], in_=contribs_sb[:, lo:hi])
                    cc_out = dram.tile([128, NE // 4], F32, tag="cc_out")
                    nc.gpsimd.collective_compute(
                        "AllReduce", OP.add, replica_groups=REPLICA_PAIRS,
                        ins=[cc_in.opt()], outs=[cc_out.opt()])
                    nc.gpsimd.dma_start(out=carry_sb[:, lo:hi], in_=cc_out[:])
                    nc.vector.tensor_mul(carry_sb[:, lo:hi], carry_sb[:, lo:hi],
                                         cmu_sb[:, lo:hi])

        if dbg_aps is not None:
            for k in range(NK):
                nc.sync.dma_start(out=dbg_aps["dbg_xq"][k * 128:(k + 1) * 128, :],
                                  in_=xq_all[:, k, :])

        # ---- phase 2: carry fixups, g matmuls, silu, h^2 column-sum,
        #      h = h*sogn*sw ----
        with tc.tile_pool(name="ps2", bufs=2, space="PSUM") as ps2, \
             tc.tile_pool(name="pss", bufs=1, space="PSUM") as pss:
            ss = []
            for th in range(NTH):
                ss_th = pss.tile([1, MV], F32, tag=f"ss{th}")
                ss.append(ss_th)
            for e in range(NE):
                ps_g = ps2.tile([128, TC], F32, tag="ps_g")
                for kg in range(NKG):
                    wg_t = wpool.tile([128, KB * 128], F16, tag="wg")
                    nc.sync.dma_start(out=wg_t[:], in_=wg[e, kg])
                    for kl in range(KB):
                        k = kg * KB + kl
                        for th in range(NTH):
                            nc.tensor.matmul(ps_g[:, th * MV:(th + 1) * MV],
                                             wg_t[:, kl * 128:(kl + 1) * 128],
                                             xq_all[:, k, th * MV:(th + 1) * MV],
                                             start=(k == 0), stop=(k == NK - 1))
                h_e = h_all[:, e, :]
                # carry fixup: h += Fprod * (carry * is_upper_half)
                nc.vector.scalar_tensor_tensor(h_e, fo_all[:, e, :],
                                               carry_sb[:, e:e + 1], h_e,
                                               OP.mult, OP.add)
                sq = work.tile([128, TC], F16, tag="sq")
                nc.scalar.activation(sq[:], h_e, AF.Square)
                if e % 2 == 1:
                    # pair-sum h^2 on the DVE and halve the ones-matmul count
                    sq2 = work.tile([128, TC], F16, tag="wkc")
                    nc.vector.tensor_tensor(sq2[:], sq_prev[:], sq[:], OP.add)
                    for th in range(NTH):
                        nc.tensor.matmul(ss[th][:], ones_sb[:],
                                         sq2[:, th * MV:(th + 1) * MV],
                                         start=(e == 1), stop=(e == NE - 1))
                sq_prev = sq
                sw = work.tile([128, TC], F32, tag="wkb")
                nc.scalar.activation(sw[:], ps_g[:], AF.Silu,
                                     scale=sg_sb[:, e:e + 1])
                # o = h*sogn*sw overwrites the Fprod slot (fp16); the rms
                # scale R commutes past the o_proj contraction and is applied
                # per-column on the PSUM output in phase 3
                nc.vector.scalar_tensor_tensor(fo_all[:, e, :], h_e,
                                               sogn_sb[:, e:e + 1],
                                               sw[:], OP.mult, OP.mult)

            if dbg_aps is not None:
                for e in range(NE):
                    nc.sync.dma_start(out=dbg_aps["dbg_h"][e * 128:(e + 1) * 128, :],
                                      in_=h_all[:, e, :])

            # 256 * rms_inv = 1/sqrt((mean + eps)/65536), broadcast across
            # partitions (the 256 pre-scales the output for fixed rounding)
            r_row = singles.tile([1, TC], F32)
            for th in range(NTH):
                nc.scalar.activation(r_row[:, th * MV:(th + 1) * MV], ss[th][:],
                                     AF.Sqrt, bias=eps_sb[:1, 0:1],
                                     scale=1.0 / (D * 65536.0))
            nc.vector.reciprocal(r_row[:], r_row[:])
            r_dram = dram.tile([1, TC], F32, tag="r_dram")
            nc.sync.dma_start(out=r_dram[:], in_=r_row[:])
            R_sb = singles.tile([128, TC], F32)
            nc.sync.dma_start(out=R_sb[:], in_=r_dram[:].to_broadcast([128, TC]))

        if dbg_aps is not None:
            nc.sync.dma_start(out=dbg_aps["dbg_r"], in_=r_row[:])
            for e in range(NE):
                nc.sync.dma_start(out=dbg_aps["dbg_hsw"][e * 128:(e + 1) * 128, :],
                                  in_=h_all[:, e, :])

        # ---- phase 3: out^T = wo.T @ o, then *R256 per column + final round ----
        xq_pool_cm.__exit__(None, None, None)
        if dbg_aps is not None:
            for e in range(NE):
                nc.sync.dma_start(out=dbg_aps["dbg_o"][e * 128:(e + 1) * 128, :],
                                  in_=fo_all[:, e, :])
        with tc.tile_pool(name="ps3", bufs=2, space="PSUM") as ps3:
            for d in range(NE):
                ps_o = ps3.tile([128, TC], F32, tag="ps_o")
                for eg in range(NKG):
                    wo_t = wpool.tile([128, KB * 128], F16, tag="wo")
                    nc.sync.dma_start(out=wo_t[:], in_=wo[d, eg])
                    for el in range(KB):
                        e = eg * KB + el
                        for th in range(NTH):
                            nc.tensor.matmul(ps_o[:, th * MV:(th + 1) * MV],
                                             wo_t[:, el * 128:(el + 1) * 128],
                                             fo_all[:, e, th * MV:(th + 1) * MV],
                                             start=(e == 0), stop=(e == NE - 1))
                # post-process per MV-wide half, entirely on the DVE queue
                # (plus a DVE-issued store) so the drain chain after the last
                # matmul crosses no engine boundary. The device emits
                # round(v*256); the exact *1/256 happens in the host gather.
                for th in range(NTH):
                    sl = slice(th * MV, (th + 1) * MV)
                    t0 = work.tile([128, MV], F32, tag="wkc")
                    nc.vector.tensor_tensor(t0[:], ps_o[:, sl], R_sb[:, sl],
                                            OP.mult)
                    ot = work.tile([128, MV], F32, tag="wkb")
                    nc.vector.tensor_scalar(ot[:], t0[:], MAGIC, MAGIC,
                                            OP.add, OP.subtract)
                    nc.scalar.dma_start(out=outT[d * 128:(d + 1) * 128, sl],
                                        in_=ot[:])
        fo_pool_cm.__exit__(None, None, None)


_NC_CACHE = None


def _get_nc():
    global _NC_CACHE
    if _NC_CACHE is None:
        _NC_CACHE = _build_kernel()
    return _NC_CACHE


def _retile(w):
    # [R, C] -> [NC_col, NKG, 128, KB*128] fp16, where
    # out[c, kg, p, kl*128 + m] = w[(kg*KB+kl)*128 + p, c*128 + m].
    # Slice [:, kl*128:(kl+1)*128] of tile (c, kg) is the lhsT for
    # contraction chunk k = kg*KB+kl and output-column chunk c.
    g = w.astype(np.float16).reshape(NKG, KB, 128, NE, 128)
    return np.ascontiguousarray(g.transpose(3, 0, 2, 1, 4).reshape(NE, NKG, 128, KB * 128))


def _scale_cols(s):
    # [D] -> [128, NE] with column e = s[e*128:(e+1)*128]
    return np.ascontiguousarray(s.reshape(NE, 128).T)


def _make_in_maps(inputs):
    hidden_states = np.asarray(inputs["hidden_states"], dtype=np.float32)
    wi_t = _retile(np.asarray(inputs["w_i"], np.float32))
    wf_t = _retile(np.asarray(inputs["w_f"], np.float32))
    wg_t = _retile(np.asarray(inputs["w_g"], np.float32))
    # o_proj: kernel reads wo[d, eg] batches; slice el is the lhsT
    # (w_o.T)[(eg*KB+el)*128 : .. , d*128 : ..]
    wo_t = _retile(np.ascontiguousarray(np.asarray(inputs["w_o"], np.float32).T))
    si_c = _scale_cols(np.asarray(inputs["s_i"], np.float32))
    sf_c = _scale_cols(np.asarray(inputs["s_f"], np.float32))
    sg_c = _scale_cols(np.asarray(inputs["s_g"], np.float32))
    sogn_c = _scale_cols(np.asarray(inputs["s_o"], np.float32)
                         * np.asarray(inputs["g_norm_w"], np.float32))

    in_maps = []
    for c in range(8):
        b, half = divmod(c, 2)
        xT = np.ascontiguousarray(
            hidden_states[b, half * TC:(half + 1) * TC, :].T)
        in_maps.append({
            "xT": xT, "wi": wi_t, "wf": wf_t, "wg": wg_t, "wo": wo_t,
            "si": si_c, "sf": sf_c, "sg": sg_c, "sogn": sogn_c,
            "cmc": np.full((128, 1), 1.0 - half, np.float32),
            "cmu": np.full((128, NE), float(half), np.float32),
        })
    return in_maps


def kernel(hidden_states, w_i, w_f, w_g, w_o, s_i, s_f, s_g, s_o, g_norm_w):
    nc = _get_nc()
    in_maps = _make_in_maps(dict(
        hidden_states=hidden_states, w_i=w_i, w_f=w_f, w_g=w_g, w_o=w_o,
        s_i=s_i, s_f=s_f, s_g=s_g, s_o=s_o, g_norm_w=g_norm_w))
    res = run_bass_kernel_spmd(nc, in_maps, list(range(8)))
    out = np.empty((B, T, D), np.float32)
    for c in range(8):
        b, half = divmod(c, 2)
        # device emits round(v*256); the *1/256 here is exact (power of two)
        out[b, half * TC:(half + 1) * TC, :] = res.results[c]["outT"].T * np.float32(1 / 256)
    return out
